# revision 3
# baseline (speedup 1.0000x reference)
"""Message-passing kernel for Trainium2 (8 NeuronCores, data-parallel over batch).

Reference computation (per batch element, C=128 channels, H=128, W=256):
  4 sequential directional scans (down, up, right, left); each scan step is
    out[i] = x[i] + relu(conv1d(out[i-1]))
  with a 'same'-padded K=9 conv1d (C->C) along the non-scan spatial axis.

Design (per core, one batch element):
  - whole image resident in SBUF as [C=128 partitions, H*260] fp32r
    (row stride 260: col 0 = zero, cols [1:257] = data, [257:260] = zero)
  - each scan step: 9 PSUM-accumulated fp32r matmuls (one per conv tap,
    weights stationary per tap, rhs = previous row/carry streamed
    contiguously, >=256-wide streams to stay on the fp32r fast path;
    per-tap psum drain offsets aligned to 8B via odd-tap rhs base shift)
  - recurrence update x + relu(psum) fused into one DVE
    scalar_tensor_tensor (max(psum,0) + x)
  - filler matmuls into a scratch psum bank keep the PE HAM-warm (2.4 GHz)
    through each step's DVE/semaphore window
  - right/left scans keep a contiguous carry tile; image columns are
    updated off the critical path by a ScalarE copy
"""

import numpy as np

C = 128
H = 128
W = 256
K = 9
RS = 260          # image row stride (fp32 words)
CT = 272          # carry tile width for right/left scans
B = 8
N_CORES = 8

_CACHE = {}


# ---------------------------------------------------------------------------
# workarounds for this walrus build (exit drain / per-instruction wait limits)
# ---------------------------------------------------------------------------

def _patch_tile_drain():
    import concourse.mybir as mybir
    import concourse.tile as tile_mod
    from concourse.vector_clock import ScopedClock

    def _drain_and_barrier(self, tick_clock, wait_clock):
        nc = self.nc
        probe = nc.sync.nop()
        wait_clock.add_sem_waits(
            probe.ins, ScopedClock({None: tick_clock.global_clock})
        )
        si = probe.ins.sync_info
        waits = list(si.on_wait) if si is not None else []
        if si is not None:
            probe.ins.sync_info = mybir.SyncInfo(
                on_wait=[], on_update=list(si.on_update)
            )
        for w in waits:
            wi = nc.sync.nop()
            wi.ins.sync_info = mybir.SyncInfo(on_wait=[w], on_update=[])
        nc.sync.drain()

        nc.all_engine_barrier()
        assert self.sems is not None
        popped = nc._tile_sem_poison_stack.pop()
        assert popped is self._sem_poison
        nc.clear_and_free_semaphores(list(self.sems.allocated().values()))
        nc.all_engine_barrier()

    tile_mod.TileContext._drain_and_barrier = _drain_and_barrier


def _split_waits(nc, max_waits=1):
    import concourse.mybir as mybir

    ctr = 0
    for f in nc.m.functions:
        for bb in f.blocks:
            insts = bb.instructions
            if not any(
                i.sync_info is not None and len(i.sync_info.on_wait) > max_waits
                for i in insts
            ):
                continue
            new = []
            for inst in insts:
                si = inst.sync_info
                ws = list(si.on_wait) if si is not None else []
                if len(ws) > max_waits:
                    extra, keep = ws[:-max_waits], ws[-max_waits:]
                    for j in range(0, len(extra), max_waits):
                        ctr += 1
                        nop = mybir.InstNoOp(
                            name=f"waitsplit-{ctr}",
                            sync_info=mybir.SyncInfo(
                                on_wait=extra[j:j + max_waits], on_update=[]
                            ),
                            bass_nofuse=True,
                            engine=inst.engine,
                        )
                        new.append(nop)
                    inst.sync_info = mybir.SyncInfo(
                        on_wait=keep, on_update=list(si.on_update)
                    )
                new.append(inst)
            bb.instructions = new


# ---------------------------------------------------------------------------
# program construction
# ---------------------------------------------------------------------------

def _build_program(n_fill=2):
    import concourse.bass as bass
    import concourse.mybir as mybir
    from concourse.alu_op_type import AluOpType
    from concourse.tile import TileContext

    _patch_tile_drain()

    f32 = mybir.dt.float32
    f32r = mybir.dt.float32r
    u32 = mybir.dt.uint32

    nc = bass.Bass()
    x_in = nc.declare_dram_parameter("x", [C, H * W], f32r, isOutput=False)
    w_in = {}
    for nm in ("wd", "wu", "wr", "wl"):
        w_in[nm] = nc.declare_dram_parameter(nm, [C, K * C], f32r, isOutput=False)
    y_out = nc.declare_dram_parameter("y", [C, H * W], f32, isOutput=True)

    with TileContext(nc) as tc:
        with (
            tc.tile_pool(name="img", bufs=1) as imgp,
            tc.tile_pool(name="wpool", bufs=1) as wp,
            tc.tile_pool(name="cpool", bufs=1) as cp,
            tc.tile_pool(name="psum", bufs=4, space="PSUM") as pp,
            tc.tile_pool(name="fpsum", bufs=2, space="PSUM") as fp,
        ):
            img = imgp.tile([C, H * RS], f32r, tag="img")
            img3 = img.rearrange("p (h r) -> p h r", r=RS)
            # zero the per-row guard columns (0 and 257..259)
            nc.vector.memset(img3[:, :, 0:1].bitcast(u32), 0)
            nc.vector.memset(img3[:, :, 257:260].bitcast(u32), 0)
            # load x into the data region, 16-row blocks
            x3 = x_in.rearrange("p (h w) -> p h w", w=W)
            for hb in range(0, H, 16):
                nc.sync.dma_start(
                    out=img3[:, hb:hb + 16, 1:257], in_=x3[:, hb:hb + 16, :]
                )

            wt = {}
            for nm in ("wd", "wu", "wr", "wl"):
                wt[nm] = wp.tile([C, K * C], f32r, tag=f"wt_{nm}", name=f"wt_{nm}")
                nc.sync.dma_start(out=wt[nm][:], in_=w_in[nm][:])

            # carry tiles for right/left scans: [0]=0, [1:129]=data,
            # [129:137]=0 (conv guard), rest finite junk
            cts = []
            for ci in range(3):
                t = cp.tile([C, CT], f32r, tag=f"ct{ci}", name=f"ct{ci}")
                nc.vector.memset(t[:].bitcast(u32), 0)
                cts.append(t)

            filler_rhs = wt["wd"][:, 0:512]

            def row(i):
                return img3[:, i, :]

            def taps(wtile, rhs_even, rhs_odd, ps, width):
                # accumulate 9 conv taps into ps; real result at ps[:, 4:4+width]
                for t in range(K):
                    s = t - 4
                    wsl = wtile[:, t * C:(t + 1) * C]
                    if s % 2 == 0:
                        nc.tensor.matmul(
                            ps[:, 4 - s:4 - s + rhs_even.shape[-1]],
                            wsl, rhs_even, start=(t == 0), stop=(t == K - 1),
                        )
                    else:
                        nc.tensor.matmul(
                            ps[:, 3 - s:3 - s + rhs_odd.shape[-1]],
                            wsl, rhs_odd, start=(t == 0), stop=(t == K - 1),
                        )

            def fillers(n):
                if not n:
                    return
                fps = fp.tile([C, 512], f32, tag="fps")
                for fi in range(n):
                    nc.tensor.matmul(
                        fps[:], wt["wd"][:, fi * C:(fi + 1) * C], filler_rhs,
                        start=(fi == 0), stop=(fi == n - 1),
                    )

            # ---------------- phase 1: down (i = 1..H-1 reads i-1) ---------
            # ---------------- phase 2: up (i = H-2..0 reads i+1) -----------
            for phase, wname, order in (
                (1, "wd", range(1, H)),
                (2, "wu", range(H - 2, -1, -1)),
            ):
                src_off = -1 if phase == 1 else 1
                for i in order:
                    r = row(i + src_off)
                    ps = pp.tile([C, 264], f32, tag="ps")
                    taps(wt[wname], r[:, 1:257], r[:, 0:258], ps, W)
                    nc.vector.scalar_tensor_tensor(
                        out=row(i)[:, 1:257], in0=ps[:, 4:260], scalar=0.0,
                        in1=row(i)[:, 1:257],
                        op0=AluOpType.max, op1=AluOpType.add,
                    )
                    fillers(n_fill)

            # ---------------- phases 3/4: right then left ------------------
            def col(w):
                # image column w: [C, H] strided (stride RS), data offset 1+w
                return img3[:, :, 1 + w]

            for phase, wname, order in (
                (3, "wr", range(1, W)),
                (4, "wl", range(W - 2, -1, -1)),
            ):
                src_off = -1 if phase == 3 else 1
                first = order.start + (-1 if phase == 3 else 1)
                carry = cts[0]
                nc.vector.tensor_copy(carry[:, 1:129], col(first))
                for n, wi in enumerate(order):
                    ps = pp.tile([C, 264], f32, tag="ps")
                    taps(wt[wname], carry[:, 1:257], carry[:, 0:258], ps, H)
                    newc = cts[(n + 1) % 3]
                    nc.vector.scalar_tensor_tensor(
                        out=newc[:, 1:129], in0=ps[:, 4:132], scalar=0.0,
                        in1=col(wi), op0=AluOpType.max, op1=AluOpType.add,
                    )
                    # persist the column for the next phase / output
                    nc.scalar.copy(col(wi), newc[:, 1:129])
                    carry = newc
                    fillers(n_fill)

            # ---------------- store --------------------------------------
            y3 = y_out.rearrange("p (h w) -> p h w", w=W)
            for hb in range(0, H, 16):
                nc.sync.dma_start(
                    out=y3[:, hb:hb + 16, :],
                    in_=img3[:, hb:hb + 16, 1:257].bitcast(f32),
                )

    _split_waits(nc, max_waits=1)
    return nc


def _get_program():
    key = "prog"
    if key not in _CACHE:
        _CACHE[key] = _build_program()
    return _CACHE[key]


# ---------------------------------------------------------------------------
# entry point
# ---------------------------------------------------------------------------

def kernel(x, w_down, w_up, w_right, w_left, _trace=False):
    from concourse.bass_utils import run_bass_kernel_spmd

    nc = _get_program()

    def prep_w(w):
        # w: (Cout, Cin, K) -> lhsT layout [Cin, K*Cout]
        return np.ascontiguousarray(
            np.transpose(np.asarray(w, np.float32), (1, 2, 0)).reshape(C, K * C)
        )

    wd, wu, wr, wl = (prep_w(w) for w in (w_down, w_up, w_right, w_left))
    x = np.asarray(x, np.float32)
    in_maps = [
        {
            "x": np.ascontiguousarray(x[b].reshape(C, H * W)),
            "wd": wd, "wu": wu, "wr": wr, "wl": wl,
        }
        for b in range(B)
    ]
    res = run_bass_kernel_spmd(
        nc, in_maps, list(range(N_CORES)), trace=_trace
    )
    out = np.stack(
        [res.results[b]["y"].reshape(C, H, W) for b in range(B)]
    ).astype(np.float32)
    if _trace:
        return out, res
    return out


# revision 4
# speedup vs baseline: 1.1170x; 1.1170x over previous
"""Message-passing kernel for Trainium2 (8 NeuronCores, data-parallel over batch).

Reference computation (per batch element, C=128 channels, H=128, W=256):
  4 sequential directional scans (down, up, right, left); each scan step is
    out[i] = x[i] + relu(conv1d(out[i-1]))
  with a 'same'-padded K=9 conv1d (C->C) along the non-scan spatial axis.

Design (per core, one batch element):
  - whole image resident in SBUF as [C=128 partitions, H*260] fp32r
    (row stride 260: col 0 = zero, cols [1:257] = data, [257:260] = zero)
  - each scan step: 9 PSUM-accumulated fp32r matmuls (one per conv tap,
    weights stationary per tap, rhs = previous row/carry streamed
    contiguously, >=256-wide streams to stay on the fp32r fast path;
    per-tap psum drain offsets kept 8B-aligned by shifting the rhs base
    through a leading zero column for odd taps)
  - recurrence update x + relu(psum) fused into one DVE
    scalar_tensor_tensor (max(psum,0) + x)
  - filler matmuls into a scratch psum bank keep the PE HAM-warm (2.4 GHz)
    through each step's DVE/semaphore window
  - right/left scans keep a contiguous carry tile; the +x columns are
    prefetched to contiguous tiles by ScalarE ahead of time; left-scan
    output is staged in w-major blocks and streamed to DRAM during the
    scan (host undoes the w-major layout)
"""

import numpy as np

C = 128
H = 128
W = 256
K = 9
RS = 260          # image row stride (fp32 words)
CT = 272          # carry tile width for right/left scans
B = 8
N_CORES = 8
SBLK = 32         # output staging block (columns)

_CACHE = {}


# ---------------------------------------------------------------------------
# workarounds for this walrus build (exit drain / per-instruction wait limits)
# ---------------------------------------------------------------------------

def _patch_tile_drain():
    import concourse.mybir as mybir
    import concourse.tile as tile_mod
    from concourse.vector_clock import ScopedClock

    def _drain_and_barrier(self, tick_clock, wait_clock):
        nc = self.nc
        probe = nc.sync.nop()
        wait_clock.add_sem_waits(
            probe.ins, ScopedClock({None: tick_clock.global_clock})
        )
        si = probe.ins.sync_info
        waits = list(si.on_wait) if si is not None else []
        if si is not None:
            probe.ins.sync_info = mybir.SyncInfo(
                on_wait=[], on_update=list(si.on_update)
            )
        for w in waits:
            wi = nc.sync.nop()
            wi.ins.sync_info = mybir.SyncInfo(on_wait=[w], on_update=[])
        nc.sync.drain()

        nc.all_engine_barrier()
        assert self.sems is not None
        popped = nc._tile_sem_poison_stack.pop()
        assert popped is self._sem_poison
        nc.clear_and_free_semaphores(list(self.sems.allocated().values()))
        nc.all_engine_barrier()

    tile_mod.TileContext._drain_and_barrier = _drain_and_barrier


def _split_waits(nc, max_waits=1):
    """This walrus build allows only one semaphore wait per instruction;
    move excess waits onto nops inserted just before, same engine.  Keep a
    PE-updated semaphore (typically the psum producer, last to arrive) on
    the instruction itself so the chained-nop latency hides behind it."""
    import concourse.mybir as mybir

    ctr = 0
    for f in nc.m.functions:
        for bb in f.blocks:
            insts = bb.instructions
            if not any(
                i.sync_info is not None and len(i.sync_info.on_wait) > max_waits
                for i in insts
            ):
                continue
            new = []
            for inst in insts:
                si = inst.sync_info
                ws = list(si.on_wait) if si is not None else []
                if len(ws) > max_waits:
                    ws.sort(key=lambda w: "PE" in (w.ant_name or ""))
                    extra, keep = ws[:-max_waits], ws[-max_waits:]
                    for j in range(0, len(extra), max_waits):
                        ctr += 1
                        nop = mybir.InstNoOp(
                            name=f"waitsplit-{ctr}",
                            sync_info=mybir.SyncInfo(
                                on_wait=extra[j:j + max_waits], on_update=[]
                            ),
                            bass_nofuse=True,
                            engine=inst.engine,
                        )
                        new.append(nop)
                    inst.sync_info = mybir.SyncInfo(
                        on_wait=keep, on_update=list(si.on_update)
                    )
                new.append(inst)
            bb.instructions = new


# ---------------------------------------------------------------------------
# program construction
# ---------------------------------------------------------------------------

def _build_program(n_fill=3):
    import concourse.bass as bass
    import concourse.mybir as mybir
    from concourse.alu_op_type import AluOpType
    from concourse.tile import TileContext

    _patch_tile_drain()

    f32 = mybir.dt.float32
    f32r = mybir.dt.float32r
    u32 = mybir.dt.uint32

    nc = bass.Bass()
    x_in = nc.declare_dram_parameter("x", [C, H * W], f32r, isOutput=False)
    w_in = {}
    for nm in ("wd", "wu", "wr", "wl"):
        w_in[nm] = nc.declare_dram_parameter(nm, [C, K * C], f32r, isOutput=False)
    # w-major output: y[c, w*H + h]; host transposes back
    y_out = nc.declare_dram_parameter("y", [C, W * H], f32, isOutput=True)

    with TileContext(nc) as tc:
        with (
            tc.tile_pool(name="img", bufs=1) as imgp,
            tc.tile_pool(name="wpool", bufs=1) as wp,
            tc.tile_pool(name="cpool", bufs=1) as cp,
            tc.tile_pool(name="stage", bufs=1) as sp,
            tc.tile_pool(name="psum", bufs=4, space="PSUM") as pp,
            tc.tile_pool(name="fpsum", bufs=2, space="PSUM") as fp,
        ):
            # weights first: the first scan stalls on them, x streams after
            wt = {}
            for nm in ("wd", "wu", "wr", "wl"):
                wt[nm] = wp.tile([C, K * C], f32r, tag=f"wt_{nm}", name=f"wt_{nm}")
                nc.sync.dma_start(out=wt[nm][:], in_=w_in[nm][:])

            img = imgp.tile([C, H * RS], f32r, tag="img")
            img3 = img.rearrange("p (h r) -> p h r", r=RS)
            # zero the per-row guard columns (0 and 257..259)
            nc.vector.memset(img3[:, :, 0:1].bitcast(u32), 0)
            nc.vector.memset(img3[:, :, 257:260].bitcast(u32), 0)
            # load x into the data region, 16-row blocks
            x3 = x_in.rearrange("p (h w) -> p h w", w=W)
            for hb in range(0, H, 16):
                nc.sync.dma_start(
                    out=img3[:, hb:hb + 16, 1:257], in_=x3[:, hb:hb + 16, :]
                )

            # carry tiles for right/left scans: [0]=0, [1:129]=data,
            # [129:137]=0 (conv guard), rest finite junk
            cts = []
            for ci in range(3):
                t = cp.tile([C, CT], f32r, tag=f"ct{ci}", name=f"ct{ci}")
                nc.vector.memset(t[:].bitcast(u32), 0)
                cts.append(t)
            # contiguous prefetched +x columns for right/left scans
            xcols = [
                cp.tile([C, C], f32r, tag=f"xc{ci}", name=f"xc{ci}")
                for ci in range(4)
            ]
            # w-major output staging blocks
            stg = [
                sp.tile([C, SBLK * H], f32, tag=f"stg{ci}", name=f"stg{ci}")
                for ci in range(2)
            ]

            filler_rhs = wt["wd"][:, 0:256]

            def row(i):
                return img3[:, i, :]

            def col(w):
                # image column w: [C, H] stride RS, data offset 1+w
                return img3[:, :, 1 + w]

            def taps(wtile, rhs_even, rhs_odd, ps):
                for t in range(K):
                    s = t - 4
                    wsl = wtile[:, t * C:(t + 1) * C]
                    if s % 2 == 0:
                        nc.tensor.matmul(
                            ps[:, 4 - s:4 - s + rhs_even.shape[-1]],
                            wsl, rhs_even, start=(t == 0), stop=(t == K - 1),
                        )
                    else:
                        nc.tensor.matmul(
                            ps[:, 3 - s:3 - s + rhs_odd.shape[-1]],
                            wsl, rhs_odd, start=(t == 0), stop=(t == K - 1),
                        )

            def fillers(n):
                if not n:
                    return
                fps = fp.tile([C, 256], f32, tag="fps")
                for fi in range(n):
                    nc.tensor.matmul(
                        fps[:], wt["wd"][:, fi * C:(fi + 1) * C], filler_rhs,
                        start=(fi == 0), stop=(fi == n - 1),
                    )

            # ---------------- phase 1 down / phase 2 up --------------------
            for phase, wname, order in (
                (1, "wd", range(1, H)),
                (2, "wu", range(H - 2, -1, -1)),
            ):
                src_off = -1 if phase == 1 else 1
                for i in order:
                    r = row(i + src_off)
                    ps = pp.tile([C, 264], f32, tag="ps")
                    taps(wt[wname], r[:, 1:257], r[:, 0:258], ps)
                    nc.vector.scalar_tensor_tensor(
                        out=row(i)[:, 1:257], in0=ps[:, 4:260], scalar=0.0,
                        in1=row(i)[:, 1:257],
                        op0=AluOpType.max, op1=AluOpType.add,
                    )
                    fillers(n_fill)

            # ---------------- phase 3: right -------------------------------
            carry = cts[0]
            nc.vector.tensor_copy(carry[:, 1:129], col(0))
            # prefetch +x columns (2 ahead)
            nc.scalar.copy(xcols[1 % 4][:], col(1))
            nc.scalar.copy(xcols[2 % 4][:], col(2))
            for w in range(1, W):
                ps = pp.tile([C, 264], f32, tag="ps")
                taps(wt["wr"], carry[:, 1:257], carry[:, 0:258], ps)
                newc = cts[w % 3]
                nc.vector.scalar_tensor_tensor(
                    out=newc[:, 1:129], in0=ps[:, 4:132], scalar=0.0,
                    in1=xcols[w % 4][:], op0=AluOpType.max, op1=AluOpType.add,
                )
                # persist for phase 4's +x reads
                nc.scalar.copy(col(w), newc[:, 1:129])
                if w + 2 < W:
                    nc.scalar.copy(xcols[(w + 2) % 4][:], col(w + 2))
                carry = newc
                fillers(n_fill)

            # ---------------- phase 4: left (stores overlap) ---------------
            def stage_ap(w):
                b = w // SBLK
                return stg[b % 2][:, (w - b * SBLK) * H:(w - b * SBLK + 1) * H]

            def flush(b):
                nc.sync.dma_start(
                    out=y_out[:, b * SBLK * H:(b + 1) * SBLK * H],
                    in_=stg[b % 2][:].bitcast(f32),
                )

            carry = cts[0]
            nc.vector.tensor_copy(carry[:, 1:129], col(W - 1))
            nc.scalar.copy(stage_ap(W - 1), col(W - 1))
            nc.scalar.copy(xcols[(W - 2) % 4][:], col(W - 2))
            nc.scalar.copy(xcols[(W - 3) % 4][:], col(W - 3))
            for w in range(W - 2, -1, -1):
                ps = pp.tile([C, 264], f32, tag="ps")
                taps(wt["wl"], carry[:, 1:257], carry[:, 0:258], ps)
                newc = cts[w % 3]
                nc.vector.scalar_tensor_tensor(
                    out=newc[:, 1:129], in0=ps[:, 4:132], scalar=0.0,
                    in1=xcols[w % 4][:], op0=AluOpType.max, op1=AluOpType.add,
                )
                nc.scalar.copy(stage_ap(w), newc[:, 1:129])
                if w - 2 >= 0:
                    nc.scalar.copy(xcols[(w - 2) % 4][:], col(w - 2))
                carry = newc
                if w % SBLK == 0:
                    flush(w // SBLK)
                fillers(n_fill)

    _split_waits(nc, max_waits=1)
    return nc


def _get_program():
    key = "prog"
    if key not in _CACHE:
        _CACHE[key] = _build_program()
    return _CACHE[key]


# ---------------------------------------------------------------------------
# entry point
# ---------------------------------------------------------------------------

def kernel(x, w_down, w_up, w_right, w_left, _trace=False):
    from concourse.bass_utils import run_bass_kernel_spmd

    nc = _get_program()

    def prep_w(w):
        # w: (Cout, Cin, K) -> lhsT layout [Cin, K*Cout]
        return np.ascontiguousarray(
            np.transpose(np.asarray(w, np.float32), (1, 2, 0)).reshape(C, K * C)
        )

    wd, wu, wr, wl = (prep_w(w) for w in (w_down, w_up, w_right, w_left))
    x = np.asarray(x, np.float32)
    in_maps = [
        {
            "x": np.ascontiguousarray(x[b].reshape(C, H * W)),
            "wd": wd, "wu": wu, "wr": wr, "wl": wl,
        }
        for b in range(B)
    ]
    res = run_bass_kernel_spmd(
        nc, in_maps, list(range(N_CORES)), trace=_trace
    )
    # y is w-major [C, W*H]; transpose back to [C, H, W]
    out = np.stack(
        [
            res.results[b]["y"].reshape(C, W, H).transpose(0, 2, 1)
            for b in range(B)
        ]
    ).astype(np.float32)
    if _trace:
        return out, res
    return out
